# revision 1
# baseline (speedup 1.0000x reference)
"""TP-8 Trainium2 Bass kernel for a LLaDA/Llama transformer block.

Shapes (hardcoded): x [2, 1024, 4096], 32 heads x 128 head_dim,
FF=12288, non-causal attention, RMSNorm + RoPE + SwiGLU.

Sharding (per sharding_hint): tensor-parallel over the 8 cores —
q/k/v/ff sharded on the output-feature axis (4 heads / 1536 ff dims per
core), wo/w_out sharded on the contraction axis.  One fp16 on-device
AllReduce (chunked) restores the residual stream after attention; the
final projection partials are summed on the host.

Device notes:
 - Activations live transposed: [D_on_partitions, tokens]; fp16
   operands everywhere on the PE (1 cyc/row, 11-bit mantissa).
 - rms_norm scale rs = exp(-0.5*ln(mean_sq + eps)) via two ACT ops;
   norm weights are folded into the adjacent weight matrices on the
   host; 1/sqrt(head_dim) is folded into wq.
 - Cross-partition sums (sum over D, softmax denominator) use an
   all-ones stationary operand on the tensor engine.
 - RoPE is applied during the PSUM eviction of the q/k projections.
 - Softmax needs no max subtraction (logits are O(5) here).
 - The MLP matmuls read the AllReduced fp16 stream directly; the norm2
   scale is folded into the ff/up PSUM evictions so only evictions are
   gated on the norm statistics.
"""

from contextlib import ExitStack

import numpy as np

import concourse.mybir as mybir
import concourse.tile as tile
from concourse import bacc
from concourse.bass_utils import run_bass_kernel_spmd

F32 = mybir.dt.float32
F16 = mybir.dt.float16
AF = mybir.ActivationFunctionType
ALU = mybir.AluOpType

N_CORES = 8
P = 128
B, T, D, FF = 2, 1024, 4096, 12288
M = B * T            # 2048 tokens
H = 128              # head dim
HALF = 64
QC = D // N_CORES    # 512 per-core q/k/v features (4 heads)
NH = QC // H         # 4 heads per core
FC = FF // N_CORES   # 1536 per-core ff features
NKP = D // P         # 32 K-tiles over D
NFT = FC // P        # 12 M-tiles over per-core FF
NDT = D // P         # 32 D-tiles
NST = T // P         # 8 sequence tiles per batch
EPS = 1e-05
AR_CHUNKS = 4


def _build():
    nc = bacc.Bacc("TRN2", target_bir_lowering=False, num_devices=N_CORES)

    xT = nc.declare_dram_parameter("xT", [D, M], F32, isOutput=False)
    xT_h = nc.declare_dram_parameter("xT_h", [D, M], F16, isOutput=False)
    css = nc.declare_dram_parameter("css", [2, P, M], F16, isOutput=False)
    wq_t = nc.declare_dram_parameter("wq_t", [NH, P, NKP, P], F16, isOutput=False)
    wk_t = nc.declare_dram_parameter("wk_t", [NH, P, NKP, P], F16, isOutput=False)
    wv_n = nc.declare_dram_parameter("wv_n", [D, QC], F16, isOutput=False)
    wo_t = nc.declare_dram_parameter("wo_t", [NH, P, NDT, P], F16, isOutput=False)
    wf_t = nc.declare_dram_parameter("wf_t", [NFT, P, NKP, P], F16, isOutput=False)
    wu_t = nc.declare_dram_parameter("wu_t", [NFT, P, NKP, P], F16, isOutput=False)
    wout_t = nc.declare_dram_parameter("wout_t", [NDT, P, NFT, P], F16, isOutput=False)
    y = nc.declare_dram_parameter("y", [D, M], F32, isOutput=True)

    with tile.TileContext(nc) as tc:
        _emit(nc, tc, xT, xT_h, css, wq_t, wk_t, wv_n, wo_t, wf_t, wu_t, wout_t, y)
    nc.compile()
    return nc


def _emit(nc, tc, xT, xT_h, css, wq_t, wk_t, wv_n, wo_t, wf_t, wu_t, wout_t, y):
    with ExitStack() as top:
        dram_pool = top.enter_context(tc.tile_pool(name="dram", bufs=1, space="DRAM"))
        const = top.enter_context(tc.tile_pool(name="const", bufs=1))

        cc_in = dram_pool.tile([D, M], F16)
        crows = (NDT // AR_CHUNKS) * P
        cc_out = [
            dram_pool.tile([crows, M], F16, addr_space="Shared", name=f"cc_out_{k}")
            for k in range(AR_CHUNKS)
        ]

        def xmid_rows(kp, cols=slice(None)):
            k, loc = divmod(kp, NDT // AR_CHUNKS)
            return cc_out[k][loc * P : (loc + 1) * P, cols]

        ones_h = const.tile([P, P], F16)
        nc.vector.memset(ones_h[:], 1.0)
        cc_sb = const.tile([P, M], F16)
        ss_sb = const.tile([P, M], F16)
        nc.sync.dma_start(out=cc_sb[:], in_=css[0])
        nc.sync.dma_start(out=ss_sb[:], in_=css[1])
        bcast1 = const.tile([P, M], F16)
        bcast2 = const.tile([P, M], F16)
        eps_sb = const.tile([P, 1], F32)
        nc.vector.memset(eps_sb[:], EPS)

        # ---- rms-norm statistics: bcast_out[:, t] = rsqrt(ms_t + eps) ----
        def norm_pass(src, bcast_out, tag):
            with ExitStack() as ph:
                sp = ph.enter_context(tc.tile_pool(name=f"norm_{tag}", bufs=1))
                pp = ph.enter_context(
                    tc.tile_pool(name=f"norm_ps_{tag}", bufs=1, space="PSUM")
                )
                ms_ps = pp.tile([P, M], F32, name=f"ms_ps_{tag}")
                for kp in range(NKP):
                    xt = sp.tile([P, M], F16, tag="xs", bufs=3, name=f"xs_{tag}_{kp}")
                    nc.sync.dma_start(out=xt[:], in_=src(kp))
                    sq = sp.tile([P, M], F16, tag="sq", bufs=3, name=f"sq_{tag}_{kp}")
                    if kp % 2 == 0:
                        nc.scalar.activation(sq[:], xt[:], AF.Square)
                    else:
                        nc.vector.tensor_mul(sq[:], xt[:], xt[:])
                    for ch in range(M // 512):
                        nc.tensor.matmul(
                            ms_ps[:, ch * 512 : (ch + 1) * 512],
                            ones_h[:],
                            sq[:, ch * 512 : (ch + 1) * 512],
                            start=(kp == 0),
                            stop=(kp == NKP - 1),
                        )
                lnt = sp.tile([P, M], F32, name=f"lnt_{tag}")
                nc.scalar.activation(
                    lnt[:], ms_ps[:], AF.Ln, bias=eps_sb[:], scale=1.0 / D
                )
                nc.scalar.activation(bcast_out[:], lnt[:], AF.Exp, scale=-0.5)

        norm_pass(lambda kp: xT_h[kp * P : (kp + 1) * P, :], bcast1, "1")

        # ------- per batch: qkv + rope + attention + o-proj partial -------
        for b in range(B):
            bs = slice(b * T, (b + 1) * T)
            with ExitStack() as bph:
                bp = bph.enter_context(tc.tile_pool(name=f"bat_{b}", bufs=1))
                qf, kf, v_sb = [], [], []
                with ExitStack() as ph:
                    xp = ph.enter_context(tc.tile_pool(name=f"xn_{b}", bufs=1))
                    sp = ph.enter_context(tc.tile_pool(name=f"qkv_{b}", bufs=1))
                    xn = []
                    for kp in range(NKP):
                        xt = sp.tile(
                            [P, T], F16, tag="xs", bufs=3, name=f"xq_{b}_{kp}"
                        )
                        nc.sync.dma_start(
                            out=xt[:], in_=xT_h[kp * P : (kp + 1) * P, bs]
                        )
                        xnk = xp.tile([P, T], F16, tag=f"xn{kp}", name=f"xn_{b}_{kp}")
                        nc.vector.tensor_mul(xnk[:], xt[:], bcast1[:, bs])
                        xn.append(xnk)

                    # q/k projections, rope fused into the eviction
                    with ExitStack() as qph:
                        qpp = qph.enter_context(
                            tc.tile_pool(name=f"qk_ps_{b}", bufs=1, space="PSUM")
                        )
                        for which, wsrc, dst in (("q", wq_t, qf), ("k", wk_t, kf)):
                            for m in range(NH):
                                wt = sp.tile(
                                    [P, NKP, P], F16, tag="wqk", bufs=3,
                                    name=f"w{which}_{b}_{m}",
                                )
                                nc.sync.dma_start(out=wt[:], in_=wsrc[m])
                                ps = qpp.tile(
                                    [P, T], F32, tag="qk_ps", bufs=2,
                                    name=f"ps{which}_{b}_{m}",
                                )
                                for ch in range(T // 512):
                                    cs = slice(ch * 512, (ch + 1) * 512)
                                    for kp in range(NKP):
                                        nc.tensor.matmul(
                                            ps[:, cs],
                                            wt[:, kp, :],
                                            xn[kp][:, cs],
                                            start=(kp == 0),
                                            stop=(kp == NKP - 1),
                                        )
                                main = sp.tile(
                                    [P, T], F16, tag="rmain", bufs=2,
                                    name=f"rm_{which}_{b}_{m}",
                                )
                                nc.vector.scalar_tensor_tensor(
                                    main[:], ps[:], 1.0, cc_sb[:, bs],
                                    ALU.mult, ALU.mult,
                                )
                                rot = sp.tile(
                                    [P, T], F16, tag="rrot", bufs=2,
                                    name=f"rr_{which}_{b}_{m}",
                                )
                                nc.vector.scalar_tensor_tensor(
                                    rot[:HALF], ps[HALF:], -1.0,
                                    ss_sb[:HALF, bs], ALU.mult, ALU.mult,
                                )
                                nc.vector.scalar_tensor_tensor(
                                    rot[HALF:], ps[:HALF], 1.0,
                                    ss_sb[HALF:, bs], ALU.mult, ALU.mult,
                                )
                                out = bp.tile(
                                    [P, T], F16, tag=f"{which}f{m}",
                                    name=f"{which}f_{b}_{m}",
                                )
                                nc.vector.tensor_add(out[:], main[:], rot[:])
                                dst.append(out)

                    # v projection, token-major: 8 concurrent accumulators
                    with ExitStack() as vph:
                        vpp = vph.enter_context(
                            tc.tile_pool(name=f"v_ps_{b}", bufs=1, space="PSUM")
                        )
                        ps_v = [
                            vpp.tile([P, QC], F32, tag=f"vps{st}",
                                     name=f"psv_{b}_{st}")
                            for st in range(NST)
                        ]
                        for kp in range(NKP):
                            wvk = sp.tile(
                                [P, QC], F16, tag="wv", bufs=3, name=f"wv_{b}_{kp}"
                            )
                            nc.sync.dma_start(
                                out=wvk[:], in_=wv_n[kp * P : (kp + 1) * P, :]
                            )
                            for st in range(NST):
                                nc.tensor.matmul(
                                    ps_v[st][:],
                                    xn[kp][:, st * P : (st + 1) * P],
                                    wvk[:],
                                    start=(kp == 0),
                                    stop=(kp == NKP - 1),
                                )
                        for st in range(NST):
                            vt = bp.tile(
                                [P, QC], F16, tag=f"v{st}", name=f"v_{b}_{st}"
                            )
                            nc.scalar.copy(vt[:], ps_v[st][:])
                            v_sb.append(vt)

                # attention per head
                attnf = []
                afp = bph.enter_context(tc.tile_pool(name=f"attnf_{b}", bufs=1))
                with ExitStack() as ah:
                    ap_ = ah.enter_context(tc.tile_pool(name=f"att_{b}", bufs=1))
                    app = ah.enter_context(
                        tc.tile_pool(name=f"att_ps_{b}", bufs=1, space="PSUM")
                    )
                    for h in range(NH):
                        den_ps = app.tile([P, T], F32, tag="den", name=f"den_{b}_{h}")
                        at_ps = app.tile([P, T], F32, tag="at", name=f"at_{b}_{h}")

                        def emit_lg(st):
                            lg_ps = app.tile(
                                [P, T], F32, tag="lg", bufs=2,
                                name=f"lg_{b}_{h}_{st}",
                            )
                            for ch in range(T // 512):
                                cs = slice(ch * 512, (ch + 1) * 512)
                                nc.tensor.matmul(
                                    lg_ps[:, cs],
                                    kf[h][:, st * P : (st + 1) * P],
                                    qf[h][:, cs],
                                    start=True,
                                    stop=True,
                                )
                            pr = ap_.tile(
                                [P, T], F16, tag="probs", bufs=4,
                                name=f"pr_{b}_{h}_{st}",
                            )
                            for ch in range(T // 512):
                                cs = slice(ch * 512, (ch + 1) * 512)
                                nc.scalar.activation(
                                    pr[:, cs], lg_ps[:, cs], AF.Exp
                                )
                            return pr

                        # software-pipelined: logits/exp of st+1 are emitted
                        # before den/pv of st so the PE has work during exp
                        prs = [None] * NST
                        prs[0] = emit_lg(0)
                        for st in range(NST):
                            if st + 1 < NST:
                                prs[st + 1] = emit_lg(st + 1)
                            pr = prs[st]
                            for ch in range(T // 512):
                                cs = slice(ch * 512, (ch + 1) * 512)
                                nc.tensor.matmul(
                                    den_ps[:, cs],
                                    ones_h[:],
                                    pr[:, cs],
                                    start=(st == 0),
                                    stop=(st == NST - 1),
                                )
                            for ch in range(T // 512):
                                cs = slice(ch * 512, (ch + 1) * 512)
                                nc.tensor.matmul(
                                    at_ps[:, cs],
                                    v_sb[st][:, h * H : (h + 1) * H],
                                    pr[:, cs],
                                    start=(st == 0),
                                    stop=(st == NST - 1),
                                )
                        af = afp.tile([P, T], F16, tag=f"af{h}", name=f"af_{b}_{h}")
                        for ch in range(T // 512):
                            cs = slice(ch * 512, (ch + 1) * 512)
                            rec = ap_.tile(
                                [P, 512], F32, tag="rec", bufs=4,
                                name=f"rec_{b}_{h}_{ch}",
                            )
                            nc.vector.reciprocal(rec[:], den_ps[:, cs])
                            nc.vector.scalar_tensor_tensor(
                                af[:, cs], at_ps[:, cs], 1.0, rec[:],
                                ALU.mult, ALU.mult,
                            )
                        attnf.append(af)

                # o-projection partial for this batch (columns = batch)
                with ExitStack() as ph:
                    sp = ph.enter_context(tc.tile_pool(name=f"op_{b}", bufs=1))
                    pp = ph.enter_context(
                        tc.tile_pool(name=f"op_ps_{b}", bufs=1, space="PSUM")
                    )
                    wo_sb = []
                    for h in range(NH):
                        wt = sp.tile(
                            [P, NDT, P], F16, tag=f"wo{h}", name=f"wo_{b}_{h}"
                        )
                        nc.sync.dma_start(out=wt[:], in_=wo_t[h])
                        wo_sb.append(wt)
                    for dt in range(NDT):
                        ps = pp.tile(
                            [P, T], F32, tag="o_ps", bufs=2, name=f"pso_{b}_{dt}"
                        )
                        for ch in range(T // 512):
                            cs = slice(ch * 512, (ch + 1) * 512)
                            for h in range(NH):
                                nc.tensor.matmul(
                                    ps[:, cs],
                                    wo_sb[h][:, dt, :],
                                    attnf[h][:, cs],
                                    start=(h == 0),
                                    stop=(h == NH - 1),
                                )
                        xt = sp.tile(
                            [P, T], F32, tag="xs3", bufs=3, name=f"xo_{b}_{dt}"
                        )
                        nc.sync.dma_start(
                            out=xt[:], in_=xT[dt * P : (dt + 1) * P, bs]
                        )
                        osb = sp.tile(
                            [P, T], F16, tag="osb", bufs=3, name=f"osb_{b}_{dt}"
                        )
                        nc.vector.scalar_tensor_tensor(
                            osb[:], xt[:], 1.0 / N_CORES, ps[:],
                            ALU.mult, ALU.add,
                        )
                        nc.sync.dma_start(
                            out=cc_in[dt * P : (dt + 1) * P, bs], in_=osb[:]
                        )
                        if b == B - 1 and (dt + 1) % (NDT // AR_CHUNKS) == 0:
                            k = (dt + 1) // (NDT // AR_CHUNKS) - 1
                            rows = slice(
                                (dt + 1 - NDT // AR_CHUNKS) * P, (dt + 1) * P
                            )
                            nc.gpsimd.collective_compute(
                                "AllReduce",
                                ALU.add,
                                replica_groups=[list(range(N_CORES))],
                                ins=[cc_in[rows, :]],
                                outs=[cc_out[k][:, :]],
                            )

        # ---------------- norm 2 ----------------
        norm_pass(xmid_rows, bcast2, "2")

        # ---------------- SwiGLU MLP (per token-half) ----------------
        for hb in range(B):
            bs = slice(hb * T, (hb + 1) * T)
            with ExitStack() as bph:
                bp = bph.enter_context(tc.tile_pool(name=f"mlpb_{hb}", bufs=1))
                hsb = []
                with ExitStack() as ph:
                    xp = ph.enter_context(tc.tile_pool(name=f"xm_{hb}", bufs=1))
                    sp = ph.enter_context(tc.tile_pool(name=f"mlp_{hb}", bufs=1))
                    pp = ph.enter_context(
                        tc.tile_pool(name=f"mlp_ps_{hb}", bufs=1, space="PSUM")
                    )
                    # fp16 AllReduced stream used directly as matmul rhs
                    xmh = []
                    for kp in range(NKP):
                        xk = xp.tile([P, T], F16, tag=f"xm{kp}", name=f"xmh_{hb}_{kp}")
                        nc.sync.dma_start(out=xk[:], in_=xmid_rows(kp, bs))
                        xmh.append(xk)

                    ffs = []
                    for m in range(NFT):
                        for which, wsrc in (("f", wf_t), ("u", wu_t)):
                            wt = sp.tile(
                                [P, NKP, P], F16, tag="wffu", bufs=3,
                                name=f"w{which}_{hb}_{m}",
                            )
                            nc.sync.dma_start(out=wt[:], in_=wsrc[m])
                            ps = pp.tile(
                                [P, T], F32, tag=f"ps_{which}", bufs=2,
                                name=f"ps{which}_{hb}_{m}",
                            )
                            for ch in range(T // 512):
                                cs = slice(ch * 512, (ch + 1) * 512)
                                for kp in range(NKP):
                                    nc.tensor.matmul(
                                        ps[:, cs],
                                        wt[:, kp, :],
                                        xmh[kp][:, cs],
                                        start=(kp == 0),
                                        stop=(kp == NKP - 1),
                                    )
                            # fold the norm2 scale into the eviction
                            nt = sp.tile(
                                [P, T], F16, tag=f"nrm_{which}", bufs=3,
                                name=f"nt{which}_{hb}_{m}",
                            )
                            nc.vector.scalar_tensor_tensor(
                                nt[:], ps[:], 1.0, bcast2[:, bs],
                                ALU.mult, ALU.mult,
                            )
                            if which == "f":
                                ft = sp.tile(
                                    [P, T], F16, tag="ffs", bufs=3,
                                    name=f"ff_{hb}_{m}",
                                )
                                nc.scalar.activation(ft[:], nt[:], AF.Silu)
                                ffs.append(ft)
                            else:
                                ht = bp.tile(
                                    [P, T], F16, tag=f"h{m}", name=f"h_{hb}_{m}"
                                )
                                nc.vector.tensor_mul(ht[:], nt[:], ffs[m][:])
                                hsb.append(ht)

                # w_out projection + residual, partial output
                with ExitStack() as ph:
                    sp = ph.enter_context(tc.tile_pool(name=f"wo2_{hb}", bufs=1))
                    pp = ph.enter_context(
                        tc.tile_pool(name=f"wo2_ps_{hb}", bufs=1, space="PSUM")
                    )
                    for dt in range(NDT):
                        wt = sp.tile(
                            [P, NFT, P], F16, tag="wot", bufs=3,
                            name=f"wot_{hb}_{dt}",
                        )
                        nc.sync.dma_start(out=wt[:], in_=wout_t[dt])
                        ps = pp.tile(
                            [P, T], F32, tag="ps_o2", bufs=2, name=f"pso2_{hb}_{dt}"
                        )
                        for ch in range(T // 512):
                            cs = slice(ch * 512, (ch + 1) * 512)
                            for m in range(NFT):
                                nc.tensor.matmul(
                                    ps[:, cs],
                                    wt[:, m, :],
                                    hsb[m][:, cs],
                                    start=(m == 0),
                                    stop=(m == NFT - 1),
                                )
                        xm = sp.tile(
                            [P, T], F16, tag="xs4", bufs=3, name=f"xm2_{hb}_{dt}"
                        )
                        nc.sync.dma_start(out=xm[:], in_=xmid_rows(dt, bs))
                        ysb = sp.tile(
                            [P, T], F32, tag="ysb", bufs=3, name=f"ysb_{hb}_{dt}"
                        )
                        nc.vector.scalar_tensor_tensor(
                            ysb[:], xm[:], 1.0 / N_CORES, ps[:], ALU.mult, ALU.add
                        )
                        nc.sync.dma_start(
                            out=y[dt * P : (dt + 1) * P, bs], in_=ysb[:]
                        )


_NC_CACHE = {}


def _get_nc():
    if "nc" not in _NC_CACHE:
        _NC_CACHE["nc"] = _build()
    return _NC_CACHE["nc"]


def _host_prep(x, sin, cos, attn_norm_w, ff_norm_w, wq, wk, wv, wo, w_ff, w_up, w_out):
    f16 = np.float16
    x2 = np.asarray(x, np.float32).reshape(M, D)
    xT = np.ascontiguousarray(x2.T)

    sinT = np.asarray(sin, np.float32).reshape(M, HALF).T
    cosT = np.asarray(cos, np.float32).reshape(M, HALF).T
    cc = np.concatenate([cosT, cosT], axis=0)
    ss = np.concatenate([sinT, sinT], axis=0)
    css = np.stack([cc, ss]).astype(f16)

    anw = np.asarray(attn_norm_w, np.float32)[:, None]
    fnw = np.asarray(ff_norm_w, np.float32)[:, None]
    wqn = (anw * np.asarray(wq, np.float32)) * (H ** -0.5)
    wkn = anw * np.asarray(wk, np.float32)
    wvn = anw * np.asarray(wv, np.float32)
    wfn = fnw * np.asarray(w_ff, np.float32)
    wun = fnw * np.asarray(w_up, np.float32)
    wo = np.asarray(wo, np.float32)
    w_out = np.asarray(w_out, np.float32)

    def mtile(w):
        # [K, F] -> [F/P, P, K/P, P] with [m, p, kp, j] = w[kp*P+p, m*P+j]
        K, F = w.shape
        return np.ascontiguousarray(
            w.reshape(K // P, P, F // P, P).transpose(2, 1, 0, 3)
        )

    in_maps = []
    for c in range(N_CORES):
        qs = slice(c * QC, (c + 1) * QC)
        fs = slice(c * FC, (c + 1) * FC)
        in_maps.append(
            {
                "xT": xT,
                "xT_h": xT.astype(f16),
                "css": css,
                "wq_t": mtile(wqn[:, qs]).astype(f16),
                "wk_t": mtile(wkn[:, qs]).astype(f16),
                "wv_n": wvn[:, qs].astype(f16),
                # [h, p, dt, j] = wo[c*QC + h*P + p, dt*P + j]
                "wo_t": np.ascontiguousarray(
                    wo[qs, :].reshape(NH, P, NDT, P)
                ).astype(f16),
                "wf_t": mtile(wfn[:, fs]).astype(f16),
                "wu_t": mtile(wun[:, fs]).astype(f16),
                "wout_t": mtile(w_out[fs, :]).astype(f16),
            }
        )
    return in_maps


def kernel(**inputs) -> np.ndarray:
    nc = _get_nc()
    in_maps = _host_prep(**inputs)
    res = run_bass_kernel_spmd(
        nc, in_maps, core_ids=list(range(N_CORES)), trace=False
    )
    acc = res.results[0]["y"].astype(np.float64)
    for c in range(1, N_CORES):
        acc += res.results[c]["y"]
    return np.ascontiguousarray(acc.T).astype(np.float32).reshape(B, T, D)



# revision 8
# speedup vs baseline: 1.3927x; 1.3927x over previous
"""TP-8 Trainium2 Bass kernel for a LLaDA/Llama transformer block (v2).

Shapes (hardcoded): x [2, 1024, 4096], 32 heads x 128 head_dim,
FF=12288, non-causal attention, RMSNorm + RoPE + SwiGLU.

Sharding: tensor-parallel over 8 cores - q/k/v/ff sharded on the
output-feature axis (4 heads / 1536 ff dims per core), wo/w_out sharded
on the contraction axis.  Per-batch fp16 AllReduce (2 x 4MB chunks per
batch) restores the residual stream; final projection partials are
summed on the host.

v2 structure:
 - Software-pipelined across the two batches: batch 0's AllReduce and
   norm2/MLP overlap batch 1's attention/o-proj and vice versa, hiding
   the collective.
 - fp8e4 DoubleRow matmuls (half-rate rows) for q/k/v/o projections,
   softmax denominator, PV, and rms-norm square-sums.  Weights carry
   power-of-two host scales (SWQ..SWO); Sa/Sv activation scales fold
   into PSUM evictions.  Logits and the MLP stay fp16.
 - rms_norm(1) folds into the projection evictions (cc*rs1 / ss*rs1 for
   q,k via rope; a per-token [P,1] scale for v built with 1-col ones
   matmuls), so projections consume raw fp8 x and the stats pass
   overlaps them.
 - The AllReduce output is read once per batch into a resident
   [128, 32, T] fp16 tile reused by norm2 stats, ff/up matmuls and the
   final residual.
"""

from contextlib import ExitStack

import numpy as np
import ml_dtypes

import concourse.mybir as mybir
import concourse.tile as tile
from concourse import bacc
from concourse.bass_utils import run_bass_kernel_spmd

F32 = mybir.dt.float32
F16 = mybir.dt.float16
F8 = mybir.dt.float8e4
AF = mybir.ActivationFunctionType
ALU = mybir.AluOpType
DR = mybir.MatmulPerfMode.DoubleRow
E4 = ml_dtypes.float8_e4m3

N_CORES = 8
P = 128
B, T, D, FF = 2, 1024, 4096, 12288
M = B * T
H = 128
HALF = 64
QC = D // N_CORES
NH = QC // H
FC = FF // N_CORES
NKP = D // P
NPR = NKP // 2
NFT = FC // P
NDT = D // P
NST = T // P
EPS = 1e-05

SWQ = 512.0
SWK = 64.0
SWV = 64.0
SWO = 32.0
SA = 16.0
SV = 4.0


def _build():
    nc = bacc.Bacc("TRN2", target_bir_lowering=False, num_devices=N_CORES)

    x8d = nc.declare_dram_parameter("x8d", [D, M], F8, isOutput=False)
    xh8 = nc.declare_dram_parameter("xh8", [D, M], F16, isOutput=False)  # x/8
    css = nc.declare_dram_parameter("css", [2, P, M], F16, isOutput=False)
    wq8 = nc.declare_dram_parameter("wq8", [NH, P, NKP, P], F8, isOutput=False)
    wk8 = nc.declare_dram_parameter("wk8", [NH, P, NKP, P], F8, isOutput=False)
    wv8 = nc.declare_dram_parameter("wv8", [P, NKP, QC], F8, isOutput=False)
    wo8 = nc.declare_dram_parameter("wo8", [P, NH, NDT, P], F8, isOutput=False)
    wf_t = nc.declare_dram_parameter("wf_t", [NFT, P, NKP, P], F16, isOutput=False)
    wu_t = nc.declare_dram_parameter("wu_t", [NFT, P, NKP, P], F16, isOutput=False)
    wout_t = nc.declare_dram_parameter("wout_t", [NDT, P, NFT, P], F16, isOutput=False)
    y = nc.declare_dram_parameter("y", [D, M], F32, isOutput=True)

    with tile.TileContext(nc) as tc:
        _emit(nc, tc, x8d, xh8, css, wq8, wk8, wv8, wo8, wf_t, wu_t, wout_t, y)
    nc.compile()
    return nc


def _emit(nc, tc, x8d, xh8, css, wq8, wk8, wv8, wo8, wf_t, wu_t, wout_t, y):
    top = ExitStack()
    with top:
        dram_pool = top.enter_context(tc.tile_pool(name="dram", bufs=1, space="DRAM"))
        const = top.enter_context(tc.tile_pool(name="const", bufs=1))

        cc_in = [dram_pool.tile([D, T], F16, name=f"cc_in_{b}") for b in range(B)]
        cc_out = [
            [
                dram_pool.tile([D // 2, T], F16, addr_space="Shared",
                               name=f"cc_out_{b}_{k}")
                for k in range(2)
            ]
            for b in range(B)
        ]

        ones8 = const.tile([P, 2, P], F8)
        nc.vector.memset(ones8[:], 1.0)
        onescol = const.tile([P, 1], F16)
        nc.vector.memset(onescol[:], SV / (SWV * P))
        eps_sb = const.tile([P, 1], F32)
        nc.vector.memset(eps_sb[:], EPS)
        bc2 = [const.tile([P, T], F16, name=f"bc2_{b}") for b in range(B)]

        # xm pool: one tag-rotated buffer; xm[1]'s DMA WAR-waits on the
        # last ff/up read of xm[0], which matches the pipeline order.
        xmp = top.enter_context(tc.tile_pool(name="xmp", bufs=1))

        # ---- first-half state; batch-alternating tiles share tags so the
        # second batch reuses the first batch's addresses (WAR-protected)
        half1 = ExitStack()
        ep = half1.enter_context(tc.tile_pool(name="half1", bufs=1))
        cc_sb = ep.tile([P, M], F16)
        ss_sb = ep.tile([P, M], F16)
        nc.sync.dma_start(out=cc_sb[:], in_=css[0])
        nc.sync.dma_start(out=ss_sb[:], in_=css[1])
        bc1 = [ep.tile([P, T], F16, tag="bc1", name=f"bc1_{b}") for b in range(B)]
        ccrs = [ep.tile([P, T], F16, tag="ccrs", name=f"ccrs_{b}") for b in range(B)]
        ssrs = [ep.tile([P, T], F16, tag="ssrs", name=f"ssrs_{b}") for b in range(B)]
        pt_sb = [ep.tile([P, NST], F32, tag="pt", name=f"pt_{b}") for b in range(B)]
        x8t = [ep.tile([P, NKP, T], F8, tag="x8", name=f"x8_{b}") for b in range(B)]

        def emit_x8_dma(b):
            bs = slice(b * T, (b + 1) * T)
            for kp in range(NKP):
                nc.sync.dma_start(
                    out=x8t[b][:, kp, :], in_=x8d[kp * P : (kp + 1) * P, bs]
                )

        # ---------------- helpers ----------------
        def stats_sq(sq_pool, src3d, ms_ps, tag, dve_only=False):
            for pi in range(NPR):
                sq = sq_pool.tile([P, 2, T], F8, tag="sq", bufs=3,
                                  name=f"sq_{tag}_{pi}")
                for j in (0, 1):
                    kp = 2 * pi + j
                    if dve_only or kp % 2 == 1:
                        nc.vector.tensor_mul(
                            sq[:, j, :], src3d[:, kp, :], src3d[:, kp, :]
                        )
                    else:
                        nc.scalar.activation(
                            sq[:, j, :], src3d[:, kp, :], AF.Square
                        )
                for ch in range(T // 512):
                    cs = slice(ch * 512, (ch + 1) * 512)
                    nc.tensor.matmul(
                        ms_ps[:, cs],
                        ones8[:],
                        sq[:, :, cs],
                        start=(pi == 0),
                        stop=(pi == NPR - 1),
                        perf_mode=DR,
                    )

        def stats_finish(sq_pool, ms_ps, bcast_out, tag):
            lnt = sq_pool.tile([P, T], F32, tag="lnt", name=f"lnt_{tag}")
            nc.scalar.activation(
                lnt[:], ms_ps[:], AF.Ln, bias=eps_sb[:], scale=1.0 / D
            )
            nc.scalar.activation(bcast_out[:], lnt[:], AF.Exp, scale=-0.5)

        def emit_pt(b, ms_ps):
            # pt[p, st] = rs1[st*P+p] * SV/SWV via 1-col ones matmuls
            for st in range(NST):
                nc.tensor.matmul(
                    ms_ps[:, st : st + 1],
                    bc1[b][:, st * P : (st + 1) * P],
                    onescol[:],
                    start=True,
                    stop=True,
                )
            nc.scalar.activation(pt_sb[b][:], ms_ps[:, 0:NST], AF.Copy)

        def emit_ccss(b):
            bs = slice(b * T, (b + 1) * T)
            nc.vector.tensor_mul(ccrs[b][:], cc_sb[:, bs], bc1[b][:])
            nc.vector.tensor_mul(ssrs[b][:], ss_sb[:, bs], bc1[b][:])

        def emit_qk(b, sp, ap, qpp, qf, kf, insert_pt, ms_ps):
            for which, wsrc, s_w, dst in (
                ("q", wq8, 1.0 / SWQ, qf),
                ("k", wk8, 1.0 / SWK, kf),
            ):
                for m in range(NH):
                    wt = sp.tile([P, NKP, P], F8, tag="wqk", bufs=3,
                                 name=f"w{which}_{b}_{m}")
                    nc.sync.dma_start(out=wt[:], in_=wsrc[m])
                    ps = qpp.tile([P, T], F32, tag="qk", bufs=2,
                                  name=f"ps{which}_{b}_{m}")
                    for ch in range(T // 512):
                        cs = slice(ch * 512, (ch + 1) * 512)
                        for pi in range(NPR):
                            nc.tensor.matmul(
                                ps[:, cs],
                                wt[:, 2 * pi : 2 * pi + 2, :],
                                x8t[b][:, 2 * pi : 2 * pi + 2, cs],
                                start=(pi == 0),
                                stop=(pi == NPR - 1),
                                perf_mode=DR,
                            )
                    if insert_pt and which == "q" and m == 0:
                        emit_pt(b, ms_ps)
                    main = sp.tile([P, T], F16, tag="rmain", bufs=2,
                                   name=f"rm_{which}_{b}_{m}")
                    nc.vector.scalar_tensor_tensor(
                        main[:], ps[:], s_w, ccrs[b][:], ALU.mult, ALU.mult
                    )
                    rot = sp.tile([P, T], F16, tag="rrot", bufs=2,
                                  name=f"rr_{which}_{b}_{m}")
                    nc.vector.scalar_tensor_tensor(
                        rot[:HALF], ps[HALF:], -s_w, ssrs[b][:HALF],
                        ALU.mult, ALU.mult,
                    )
                    nc.vector.scalar_tensor_tensor(
                        rot[HALF:], ps[:HALF], s_w, ssrs[b][HALF:],
                        ALU.mult, ALU.mult,
                    )
                    out = ap.tile([P, T], F16, tag=f"{which}f{m}",
                                  name=f"{which}f_{b}_{m}")
                    nc.vector.tensor_add(out[:], main[:], rot[:])
                    dst[m] = out

        def emit_v(b, sp, vpp, v8_sb):
            wv_sb = sp.tile([P, NKP, QC], F8, tag="wv", name=f"wv_{b}")
            nc.sync.dma_start(out=wv_sb[:], in_=wv8[:])
            for st in range(NST):
                ps = vpp.tile([P, QC], F32, tag="vps", bufs=2,
                              name=f"psv_{b}_{st}")
                for pi in range(NPR):
                    nc.tensor.matmul(
                        ps[:],
                        x8t[b][:, 2 * pi : 2 * pi + 2, st * P : (st + 1) * P],
                        wv_sb[:, 2 * pi : 2 * pi + 2, :],
                        start=(pi == 0),
                        stop=(pi == NPR - 1),
                        perf_mode=DR,
                    )
                nc.scalar.activation(
                    v8_sb[:, st, :], ps[:], AF.Copy,
                    scale=pt_sb[b][:, st : st + 1],
                )

        def emit_attn_head(b, h, ap_, app, qf, kf, v8_sb, af8):
            den_ps = app.tile([P, T], F32, tag="den", name=f"den_{b}_{h}")
            at_ps = app.tile([P, T], F32, tag="at", name=f"at_{b}_{h}")
            pr8 = ap_.tile([P, NST, T], F8, tag="pr8", bufs=2, name=f"pr_{b}_{h}")

            def emit_lg(st):
                lg_ps = app.tile([P, T], F32, tag="lg", bufs=2,
                                 name=f"lg_{b}_{h}_{st}")
                for ch in range(T // 512):
                    cs = slice(ch * 512, (ch + 1) * 512)
                    nc.tensor.matmul(
                        lg_ps[:, cs],
                        kf[h][:, st * P : (st + 1) * P],
                        qf[h][:, cs],
                        start=True,
                        stop=True,
                    )
                nc.scalar.activation(pr8[:, st, :], lg_ps[:], AF.Exp)

            emit_lg(0)
            emit_lg(1)
            for u in range(NST // 2):
                if 2 * u + 2 < NST:
                    emit_lg(2 * u + 2)
                    emit_lg(2 * u + 3)
                for ch in range(T // 512):
                    cs = slice(ch * 512, (ch + 1) * 512)
                    nc.tensor.matmul(
                        den_ps[:, cs],
                        ones8[:],
                        pr8[:, 2 * u : 2 * u + 2, cs],
                        start=(u == 0),
                        stop=(u == NST // 2 - 1),
                        perf_mode=DR,
                    )
                for ch in range(T // 512):
                    cs = slice(ch * 512, (ch + 1) * 512)
                    nc.tensor.matmul(
                        at_ps[:, cs],
                        v8_sb[:, 2 * u : 2 * u + 2, h * H : (h + 1) * H],
                        pr8[:, 2 * u : 2 * u + 2, cs],
                        start=(u == 0),
                        stop=(u == NST // 2 - 1),
                        perf_mode=DR,
                    )
            for ch in range(T // 512):
                cs = slice(ch * 512, (ch + 1) * 512)
                rec = ap_.tile([P, 512], F32, tag="rec", bufs=4,
                               name=f"rec_{b}_{h}_{ch}")
                nc.vector.reciprocal(rec[:], den_ps[:, cs])
                nc.vector.scalar_tensor_tensor(
                    af8[:, h, cs], at_ps[:, cs], SA / SV, rec[:],
                    ALU.mult, ALU.mult,
                )

        def emit_oproj(b, sp, opp, af8, extra=None):
            bs = slice(b * T, (b + 1) * T)
            wo_sb = sp.tile([P, NH, NDT, P], F8, tag="wo", name=f"wo_{b}")
            nc.sync.dma_start(out=wo_sb[:], in_=wo8[:])
            for dt in range(NDT):
                ps = opp.tile([P, T], F32, tag="ops", bufs=2,
                              name=f"pso_{b}_{dt}")
                for ch in range(T // 512):
                    cs = slice(ch * 512, (ch + 1) * 512)
                    for u in range(NH // 2):
                        nc.tensor.matmul(
                            ps[:, cs],
                            wo_sb[:, 2 * u : 2 * u + 2, dt, :],
                            af8[:, 2 * u : 2 * u + 2, cs],
                            start=(u == 0),
                            stop=(u == NH // 2 - 1),
                            perf_mode=DR,
                        )
                xh = sp.tile([P, T], F16, tag="xh", bufs=3, name=f"xh_{b}_{dt}")
                nc.sync.dma_start(out=xh[:], in_=xh8[dt * P : (dt + 1) * P, bs])
                osb = sp.tile([P, T], F16, tag="osb", bufs=3,
                              name=f"osb_{b}_{dt}")
                nc.vector.scalar_tensor_tensor(
                    osb[:], ps[:], 1.0 / (SA * SWO), xh[:], ALU.mult, ALU.add
                )
                nc.sync.dma_start(
                    out=cc_in[b][dt * P : (dt + 1) * P, :], in_=osb[:]
                )
                if (dt + 1) % (NDT // 2) == 0:
                    k = (dt + 1) // (NDT // 2) - 1
                    rows = slice((dt + 1 - NDT // 2) * P, (dt + 1) * P)
                    nc.gpsimd.collective_compute(
                        "AllReduce",
                        ALU.add,
                        replica_groups=[list(range(N_CORES))],
                        ins=[cc_in[b][rows, :]],
                        outs=[cc_out[b][k][:, :]],
                    )
                if extra is not None and dt == 0:
                    extra()

        xm = [None, None]

        def emit_xm_dma(b, pool):
            xm[b] = pool.tile([P, NKP, T], F16, tag="xm", bufs=1, name=f"xm_{b}")
            for kp in range(NKP):
                nc.sync.dma_start(
                    out=xm[b][:, kp, :],
                    in_=cc_out[b][kp // 16][(kp % 16) * P : (kp % 16 + 1) * P, :],
                )

        def emit_mlp(b, sp, hp):
            hsb = []
            ffs = None
            with ExitStack() as psc:
                pp = psc.enter_context(
                    tc.tile_pool(name=f"mlpp{b}", bufs=1, space="PSUM")
                )
                for m in range(NFT):
                    for which, wsrc in (("f", wf_t), ("u", wu_t)):
                        wt = sp.tile([P, NKP, P], F16, tag="wffu", bufs=2,
                                     name=f"w{which}_{b}_{m}")
                        nc.sync.dma_start(out=wt[:], in_=wsrc[m])
                        ps = pp.tile([P, T], F32, tag="psfu", bufs=2,
                                     name=f"ps{which}_{b}_{m}")
                        for ch in range(T // 512):
                            cs = slice(ch * 512, (ch + 1) * 512)
                            for kp in range(NKP):
                                nc.tensor.matmul(
                                    ps[:, cs],
                                    wt[:, kp, :],
                                    xm[b][:, kp, cs],
                                    start=(kp == 0),
                                    stop=(kp == NKP - 1),
                                )
                        nt = sp.tile([P, T], F16, tag=f"nt_{which}", bufs=2,
                                     name=f"nt{which}_{b}_{m}")
                        nc.vector.scalar_tensor_tensor(
                            nt[:], ps[:], 1.0, bc2[b][:], ALU.mult, ALU.mult
                        )
                        if which == "f":
                            ffs = sp.tile([P, T], F16, tag="ffs", bufs=2,
                                          name=f"ff_{b}_{m}")
                            nc.scalar.activation(ffs[:], nt[:], AF.Silu)
                        else:
                            ht = hp.tile([P, T], F16, tag=f"h{m}",
                                         name=f"h_{b}_{m}")
                            nc.vector.tensor_mul(ht[:], nt[:], ffs[:])
                            hsb.append(ht)
            return hsb

        def emit_wout(b, sp, pp, hsb):
            bs = slice(b * T, (b + 1) * T)
            for dt in range(NDT):
                wt = sp.tile([P, NFT, P], F16, tag="wot", bufs=3,
                             name=f"wot_{b}_{dt}")
                nc.sync.dma_start(out=wt[:], in_=wout_t[dt])
                ps = pp.tile([P, T], F32, tag="pso2", bufs=2,
                             name=f"pso2_{b}_{dt}")
                for ch in range(T // 512):
                    cs = slice(ch * 512, (ch + 1) * 512)
                    for m in range(NFT):
                        nc.tensor.matmul(
                            ps[:, cs],
                            wt[:, m, :],
                            hsb[m][:, cs],
                            start=(m == 0),
                            stop=(m == NFT - 1),
                        )
                xr = sp.tile([P, T], F16, tag="xr", bufs=3,
                             name=f"xr_{b}_{dt}")
                nc.sync.dma_start(
                    out=xr[:],
                    in_=cc_out[b][dt // 16][(dt % 16) * P : (dt % 16 + 1) * P, :],
                )
                ysb = sp.tile([P, T], F32, tag="ysb", bufs=3,
                              name=f"ysb_{b}_{dt}")
                nc.vector.scalar_tensor_tensor(
                    ysb[:], xr[:], 0.125, ps[:], ALU.mult, ALU.add
                )
                nc.sync.dma_start(out=y[dt * P : (dt + 1) * P, bs], in_=ysb[:])

        # ================= emission schedule =================
        emit_x8_dma(0)

        qf = [[None] * NH for _ in range(B)]
        kf = [[None] * NH for _ in range(B)]
        v8_sb = [ep.tile([P, NST, QC], F8, tag="v8", name=f"v8_{b}")
                 for b in range(B)]
        af8 = [ep.tile([P, NH, T], F8, tag="af8", name=f"af8_{b}")
               for b in range(B)]

        # ---- b0 projections ----
        with ExitStack() as ph:
            sp = ph.enter_context(tc.tile_pool(name="prj0", bufs=1))
            stp = ph.enter_context(tc.tile_pool(name="st0", bufs=1, space="PSUM"))
            qpp = ph.enter_context(tc.tile_pool(name="qk0", bufs=1, space="PSUM"))
            vpp = ph.enter_context(tc.tile_pool(name="v0", bufs=1, space="PSUM"))
            ms_ps = stp.tile([P, T], F32, name="ms_0")
            stats_sq(sp, x8t[0], ms_ps, "n1b0")
            stats_finish(sp, ms_ps, bc1[0], "n1b0")
            emit_ccss(0)
            emit_qk(0, sp, ep, qpp, qf[0], kf[0], True, ms_ps)
            emit_v(0, sp, vpp, v8_sb[0])

        # ---- b0 attention (+ b1 x8 prefetch) ----
        with ExitStack() as ah:
            ap_ = ah.enter_context(tc.tile_pool(name="attb0", bufs=1))
            app = ah.enter_context(tc.tile_pool(name="attp0", bufs=1, space="PSUM"))
            emit_attn_head(0, 0, ap_, app, qf[0], kf[0], v8_sb[0], af8[0])
            emit_x8_dma(1)
            for h in range(1, NH):
                emit_attn_head(0, h, ap_, app, qf[0], kf[0], v8_sb[0], af8[0])

        # ---- b1 stats (DVE squares) + b0 o-proj + AR0/AR1 ----
        with ExitStack() as sh:
            sp1 = sh.enter_context(tc.tile_pool(name="st1s", bufs=1))
            stp1 = sh.enter_context(tc.tile_pool(name="st1p", bufs=1, space="PSUM"))
            ms1_ps = stp1.tile([P, T], F32, name="ms_1")
            stats_sq(sp1, x8t[1], ms1_ps, "n1b1", dve_only=True)
            stats_finish(sp1, ms1_ps, bc1[1], "n1b1")
            emit_ccss(1)
            emit_pt(1, ms1_ps)

            with ExitStack() as oh:
                osp = oh.enter_context(tc.tile_pool(name="op0", bufs=1))
                opp = oh.enter_context(
                    tc.tile_pool(name="opp0", bufs=1, space="PSUM")
                )
                emit_oproj(0, osp, opp, af8[0])

        # ---- b1 q/k/v ----
        with ExitStack() as ph:
            sp = ph.enter_context(tc.tile_pool(name="prj1", bufs=1))
            qpp = ph.enter_context(tc.tile_pool(name="qk1", bufs=1, space="PSUM"))
            vpp = ph.enter_context(tc.tile_pool(name="v1", bufs=1, space="PSUM"))
            emit_qk(1, sp, ep, qpp, qf[1], kf[1], False, None)
            emit_v(1, sp, vpp, v8_sb[1])

        # ---- b1 attention (+ b0 xm prefetch) ----
        with ExitStack() as ah:
            ap_ = ah.enter_context(tc.tile_pool(name="attb1", bufs=1))
            app = ah.enter_context(tc.tile_pool(name="attp1", bufs=1, space="PSUM"))
            emit_attn_head(1, 0, ap_, app, qf[1], kf[1], v8_sb[1], af8[1])
            emit_xm_dma(0, xmp)
            for h in range(1, NH):
                emit_attn_head(1, h, ap_, app, qf[1], kf[1], v8_sb[1], af8[1])

        # ---- b1 o-proj + AR2/AR3; b0 norm2 interleaved ----
        with ExitStack() as oh:
            osp = oh.enter_context(tc.tile_pool(name="op1", bufs=1))
            opp = oh.enter_context(tc.tile_pool(name="opp1", bufs=1, space="PSUM"))
            stp = oh.enter_context(tc.tile_pool(name="st2p0", bufs=1, space="PSUM"))
            sp2 = oh.enter_context(tc.tile_pool(name="st2s0", bufs=1))
            ms2_ps = stp.tile([P, T], F32, name="ms2_0")

            def b0_norm2():
                stats_sq(sp2, xm[0], ms2_ps, "n2b0", dve_only=True)
                stats_finish(sp2, ms2_ps, bc2[0], "n2b0")

            emit_oproj(1, osp, opp, af8[1], extra=b0_norm2)

        half1.close()

        # ---- b0 MLP ----
        with ExitStack() as mh:
            sp = mh.enter_context(tc.tile_pool(name="mlp0", bufs=1))
            hp = mh.enter_context(tc.tile_pool(name="h0", bufs=1))
            hsb = emit_mlp(0, sp, hp)

            # prefetch b1 xm + b1 norm2, then b0 wout
            emit_xm_dma(1, xmp)
            with ExitStack() as sh:
                sp2 = sh.enter_context(tc.tile_pool(name="st2s1", bufs=1))
                stp = sh.enter_context(
                    tc.tile_pool(name="st2p1", bufs=1, space="PSUM")
                )
                ms2_ps = stp.tile([P, T], F32, name="ms2_1")
                stats_sq(sp2, xm[1], ms2_ps, "n2b1", dve_only=True)
                stats_finish(sp2, ms2_ps, bc2[1], "n2b1")

                with ExitStack() as wh:
                    wsp = wh.enter_context(tc.tile_pool(name="wo2_0", bufs=1))
                    wpp = wh.enter_context(
                        tc.tile_pool(name="wo2p0", bufs=1, space="PSUM")
                    )
                    emit_wout(0, wsp, wpp, hsb)

        # ---- b1 MLP ----
        with ExitStack() as mh:
            sp = mh.enter_context(tc.tile_pool(name="mlp1", bufs=1))
            hp = mh.enter_context(tc.tile_pool(name="h1", bufs=1))
            hsb = emit_mlp(1, sp, hp)
            with ExitStack() as wh:
                wsp = wh.enter_context(tc.tile_pool(name="wo2_1", bufs=1))
                wpp = wh.enter_context(
                    tc.tile_pool(name="wo2p1", bufs=1, space="PSUM")
                )
                emit_wout(1, wsp, wpp, hsb)


_NC_CACHE = {}


def _get_nc():
    if "nc" not in _NC_CACHE:
        _NC_CACHE["nc"] = _build()
    return _NC_CACHE["nc"]


def _host_prep(x, sin, cos, attn_norm_w, ff_norm_w, wq, wk, wv, wo, w_ff, w_up, w_out):
    f16 = np.float16
    x2 = np.asarray(x, np.float32).reshape(M, D)
    xT = np.ascontiguousarray(x2.T)

    sinT = np.asarray(sin, np.float32).reshape(M, HALF).T
    cosT = np.asarray(cos, np.float32).reshape(M, HALF).T
    cc = np.concatenate([cosT, cosT], axis=0)
    ss = np.concatenate([sinT, sinT], axis=0)
    css = np.stack([cc, ss]).astype(f16)

    anw = np.asarray(attn_norm_w, np.float32)[:, None]
    fnw = np.asarray(ff_norm_w, np.float32)[:, None]
    wqn = (anw * np.asarray(wq, np.float32)) * (H ** -0.5) * SWQ
    wkn = anw * np.asarray(wk, np.float32) * SWK
    wvn = anw * np.asarray(wv, np.float32) * SWV
    won = np.asarray(wo, np.float32) * SWO
    wfn = fnw * np.asarray(w_ff, np.float32)
    wun = fnw * np.asarray(w_up, np.float32)
    w_out = np.asarray(w_out, np.float32)

    def mtile(w):
        # [K, F] -> [F/P, P, K/P, P] with [m, p, kp, j] = w[kp*P+p, m*P+j]
        K, F = w.shape
        return np.ascontiguousarray(
            w.reshape(K // P, P, F // P, P).transpose(2, 1, 0, 3)
        )

    x8_full = xT.astype(E4)
    xh8_full = (xT * 0.125).astype(f16)

    in_maps = []
    for c in range(N_CORES):
        qs = slice(c * QC, (c + 1) * QC)
        fs = slice(c * FC, (c + 1) * FC)
        in_maps.append(
            {
                "x8d": x8_full,
                "xh8": xh8_full,
                "css": css,
                "wq8": mtile(wqn[:, qs]).astype(E4),
                "wk8": mtile(wkn[:, qs]).astype(E4),
                # [p, kp, f] = wvn[kp*P+p, f]
                "wv8": np.ascontiguousarray(
                    wvn[:, qs].reshape(NKP, P, QC).transpose(1, 0, 2)
                ).astype(E4),
                # [p, h, dt, j] = wo[c*QC + h*P + p, dt*P + j]
                "wo8": np.ascontiguousarray(
                    won[qs, :].reshape(NH, P, NDT, P).transpose(1, 0, 2, 3)
                ).astype(E4),
                "wf_t": mtile(wfn[:, fs]).astype(f16),
                "wu_t": mtile(wun[:, fs]).astype(f16),
                "wout_t": mtile(w_out[fs, :]).astype(f16),
            }
        )
    return in_maps


def kernel(**inputs) -> np.ndarray:
    nc = _get_nc()
    in_maps = _host_prep(**inputs)
    res = run_bass_kernel_spmd(
        nc, in_maps, core_ids=list(range(N_CORES)), trace=False
    )
    acc = res.results[0]["y"].astype(np.float64)
    for c in range(1, N_CORES):
        acc += res.results[c]["y"]
    return np.ascontiguousarray(acc.T).astype(np.float32).reshape(B, T, D)
